# revision 1
# baseline (speedup 1.0000x reference)
"""GCN 2-layer + global mean pool on 8 Trainium2 NeuronCores.

Strategy (sharding_hint: partition nodes+incident edges, replicate weights):
- h = x@W1 computed redundantly on every core (bf16, host passes x^T).
- Layer-1 message passing partitioned by DST node: core c owns dst nodes
  [c*12500, (c+1)*12500). Edges sorted by dst-window (128 dsts/window),
  padded to C_MAX chunks of 128 edges per window. Messages gathered from
  the local h table via big indirect DMAs; scatter done as one-hot matmul
  (onehot[e, j] = (drel_e == j) * norm_e) accumulating in PSUM per window,
  output in transposed form [64f x 128dst] so bias+ReLU fuse into one
  scalar-engine activation with per-partition bias.
- h2 = relu(out1)@W2 per window on-core.
- Layer-2 message passing + mean-pool numerator folded into a dense matmul:
  pool[g,f] = sum_src Wp[src,g] * h2[src,f], where Wp[src,g] =
  sum_{edges e with src_e=src} norm_e * [batch[dst_e]=g] is precomputed on
  the host (layer-2 edges partitioned by SRC so h2 is local). Each core
  returns a partial [64,32]; host sums partials, divides by counts, adds b2.
"""

import numpy as np
import ml_dtypes

import concourse.bass as bass
import concourse.bacc as bacc
import concourse.tile as tile
import concourse.mybir as mybir
from concourse.bass_utils import run_bass_kernel_spmd
from concourse.masks import make_identity

BF16 = ml_dtypes.bfloat16

N_NODES = 100000
IN_DIM = 128
HID = 64
OUT = 32
N_GRAPHS = 64
N_CORES = 8
NSH = N_NODES // N_CORES          # 12500 dst nodes per core
N_WINDOWS = 100                   # 100 windows x 128 dst slots = 12800 >= 12500
WIN_SLOTS = N_WINDOWS * 128       # 12800
CALL_WINDOWS = 2                  # windows gathered per indirect DMA call
N_CALLS = N_WINDOWS // CALL_WINDOWS  # 50
HGROUP = 2048                     # nodes per h-phase group
N_HGROUPS = 49                    # 49*2048 = 100352
N_PAD = N_HGROUPS * HGROUP        # padded h-table rows

LAST_RESULTS = None


def _build(c_max):
    cc_n = CALL_WINDOWS * c_max   # chunks per gather call
    f32 = mybir.dt.float32
    bf16 = mybir.dt.bfloat16
    i32 = mybir.dt.int32
    RELU = mybir.ActivationFunctionType.Relu
    COPY = mybir.ActivationFunctionType.Copy

    nc = bacc.Bacc("TRN2", target_bir_lowering=False, debug=False,
                   enable_asserts=False, num_devices=N_CORES)

    xt = nc.dram_tensor("xt", [128, N_PAD], bf16, kind="ExternalInput")
    w1 = nc.dram_tensor("w1", [IN_DIM, HID], bf16, kind="ExternalInput")
    w2 = nc.dram_tensor("w2", [HID, OUT], bf16, kind="ExternalInput")
    b1 = nc.dram_tensor("b1", [HID, 1], f32, kind="ExternalInput")
    iota_in = nc.dram_tensor("iota", [128, 128], bf16, kind="ExternalInput")
    idx_in = nc.dram_tensor("idx", [N_CALLS, 128, cc_n], i32, kind="ExternalInput")
    drel_in = nc.dram_tensor("drel", [N_CALLS, 128, cc_n], f32, kind="ExternalInput")
    norm_in = nc.dram_tensor("norm", [N_CALLS, 128, cc_n], f32, kind="ExternalInput")
    wp_in = nc.dram_tensor("wp", [WIN_SLOTS, N_GRAPHS], f32, kind="ExternalInput")
    out_t = nc.dram_tensor("pool_part", [N_GRAPHS, OUT], f32, kind="ExternalOutput")

    with tile.TileContext(nc) as tc:
        with tc.tile_pool(name="const", bufs=1) as constp, \
             tc.tile_pool(name="stream", bufs=3) as sb, \
             tc.tile_pool(name="oh", bufs=6) as ohp, \
             tc.tile_pool(name="psh", bufs=2, space="PSUM") as psh, \
             tc.tile_pool(name="psr", bufs=3, space="PSUM") as psr, \
             tc.tile_pool(name="ps1", bufs=1, space="PSUM") as ps1, \
             tc.tile_pool(name="dram", bufs=1, space="DRAM") as dp:

            h_dram = dp.tile([N_PAD, HID], bf16)

            w1_sb = constp.tile([IN_DIM, HID], bf16)
            nc.sync.dma_start(w1_sb[:], w1[:])
            w2_sb = constp.tile([HID, OUT], bf16)
            nc.sync.dma_start(w2_sb[:], w2[:])
            b1_sb = constp.tile([HID, 1], f32)
            nc.sync.dma_start(b1_sb[:], b1[:])
            iota_sb = constp.tile([128, 128], bf16)
            nc.sync.dma_start(iota_sb[:], iota_in[:])
            ident = constp.tile([128, 128], f32)
            make_identity(nc, ident[:])
            h2_sb = constp.tile([128, N_WINDOWS * OUT], f32)
            wp_sb = constp.tile([128, N_WINDOWS * N_GRAPHS], f32)
            nc.sync.dma_start(
                wp_sb[:].rearrange("p (w g) -> p w g", g=N_GRAPHS),
                wp_in[:].rearrange("(w p) g -> p w g", p=128))

            # ---- phase H: h = x @ W1 for all nodes (redundant per core) ----
            kpg = HGROUP // 128
            for g in range(N_HGROUPS):
                xt_t = sb.tile([128, HGROUP], bf16, tag="xt")
                nc.sync.dma_start(xt_t[:], xt[:, g * HGROUP:(g + 1) * HGROUP])
                hstage = sb.tile([128, kpg * HID], bf16, tag="hstage")
                for k in range(kpg):
                    ph = psh.tile([128, HID], f32, tag="ph")
                    nc.tensor.matmul(ph[:], lhsT=xt_t[:, k * 128:(k + 1) * 128],
                                     rhs=w1_sb[:], start=True, stop=True)
                    nc.scalar.activation(hstage[:, k * HID:(k + 1) * HID], ph[:], COPY)
                nc.sync.dma_start(
                    h_dram[g * HGROUP:(g + 1) * HGROUP, :].rearrange(
                        "(k p) f -> p k f", p=128),
                    hstage[:].rearrange("p (k f) -> p k f", f=HID))

            # ---- phase L1: gather + one-hot matmul scatter, then h2 ----
            for call in range(N_CALLS):
                idx_sb = sb.tile([128, cc_n], i32, tag="idx")
                nc.sync.dma_start(idx_sb[:], idx_in[call, :, :])
                drel_sb = sb.tile([128, cc_n], f32, tag="drel")
                nc.sync.dma_start(drel_sb[:], drel_in[call, :, :])
                norm_sb = sb.tile([128, cc_n], f32, tag="nrm")
                nc.sync.dma_start(norm_sb[:], norm_in[call, :, :])
                gath = sb.tile([128, cc_n * HID], bf16, tag="gath")
                nc.gpsimd.indirect_dma_start(
                    out=gath[:], out_offset=None,
                    in_=h_dram[:],
                    in_offset=bass.IndirectOffsetOnAxis(ap=idx_sb[:], axis=0))
                for wl in range(CALL_WINDOWS):
                    w = call * CALL_WINDOWS + wl
                    pR = psr.tile([HID, 128], f32, tag="pR")
                    for ci in range(c_max):
                        ch = wl * c_max + ci
                        oh = ohp.tile([128, 128], bf16, tag="oh")
                        nc.vector.tensor_scalar(
                            out=oh[:], in0=iota_sb[:],
                            scalar1=drel_sb[:, ch:ch + 1],
                            scalar2=norm_sb[:, ch:ch + 1],
                            op0=mybir.AluOpType.is_equal,
                            op1=mybir.AluOpType.mult)
                        nc.tensor.matmul(pR[:], lhsT=gath[:, ch * HID:(ch + 1) * HID],
                                         rhs=oh[:], start=(ci == 0),
                                         stop=(ci == c_max - 1))
                    relu_sb = sb.tile([HID, 128], bf16, tag="relu")
                    nc.scalar.activation(relu_sb[:], pR[:], RELU, bias=b1_sb[:, 0:1])
                    ph2 = ps1.tile([OUT, 128], f32, tag="ph2")
                    nc.tensor.matmul(ph2[:], lhsT=w2_sb[:], rhs=relu_sb[:],
                                     start=True, stop=True)
                    h2t = sb.tile([OUT, 128], f32, tag="h2t")
                    nc.scalar.activation(h2t[:], ph2[:], COPY)
                    ph2t = ps1.tile([128, OUT], f32, tag="ph2t")
                    nc.tensor.transpose(ph2t[:], h2t[:], ident[:OUT, :OUT])
                    nc.vector.tensor_copy(h2_sb[:, w * OUT:(w + 1) * OUT], ph2t[:])

            # ---- phase pool: pool_part = Wp^T @ h2 ----
            pp = ps1.tile([N_GRAPHS, OUT], f32, tag="pp")
            for w in range(N_WINDOWS):
                nc.tensor.matmul(pp[:],
                                 lhsT=wp_sb[:, w * N_GRAPHS:(w + 1) * N_GRAPHS],
                                 rhs=h2_sb[:, w * OUT:(w + 1) * OUT],
                                 start=(w == 0), stop=(w == N_WINDOWS - 1))
            outsb = sb.tile([N_GRAPHS, OUT], f32, tag="out")
            nc.vector.tensor_copy(outsb[:], pp[:])
            nc.sync.dma_start(out_t[:], outsb[:])

    nc.compile()
    return nc


def kernel(x, edge_index, batch, W1, b1, W2, b2):
    global LAST_RESULTS
    x = np.asarray(x, np.float32)
    src = np.asarray(edge_index[0], np.int64)
    dst = np.asarray(edge_index[1], np.int64)
    batch = np.asarray(batch, np.int64)
    W1 = np.asarray(W1, np.float32)
    b1 = np.asarray(b1, np.float32)
    W2 = np.asarray(W2, np.float32)
    b2 = np.asarray(b2, np.float32)

    loop = np.arange(N_NODES, dtype=np.int64)
    src = np.concatenate([src, loop])
    dst = np.concatenate([dst, loop])
    deg = np.bincount(dst, minlength=N_NODES).astype(np.float32)
    dinv = 1.0 / np.sqrt(np.maximum(deg, 1.0))
    norm = (dinv[src] * dinv[dst]).astype(np.float32)
    gid = batch[dst]                      # graph id of each edge's dst

    # ---- per-core L1 edge streams (partitioned by dst) ----
    core_of_dst = dst // NSH
    per_core = []
    c_max = 1
    for c in range(N_CORES):
        m = core_of_dst == c
        s_c = src[m].astype(np.int64)
        d_loc = (dst[m] - c * NSH).astype(np.int64)
        n_c = norm[m]
        # Balance window edge counts: assign dsts to window slots snake-wise
        # in descending-degree order, so every window gets ~equal edge work.
        degc = np.bincount(d_loc, minlength=NSH)
        rank_order = np.argsort(-degc, kind="stable")
        q, rem = np.divmod(np.arange(NSH), N_WINDOWS)
        win_of_rank = np.where(q % 2 == 0, rem, N_WINDOWS - 1 - rem)
        slot_of_rank = win_of_rank * 128 + q
        slot_of_dst = np.empty(NSH, np.int64)
        slot_of_dst[rank_order] = slot_of_rank
        slot = slot_of_dst[d_loc]
        win = slot >> 7
        order = np.argsort(win, kind="stable")
        s_c, slot, n_c, win = s_c[order], slot[order], n_c[order], win[order]
        counts = np.bincount(win, minlength=N_WINDOWS)
        c_max = max(c_max, int(np.ceil(counts.max() / 128)))
        per_core.append((s_c, slot, n_c, win, counts, slot_of_dst))

    cc_n = CALL_WINDOWS * c_max
    slots_per_win = c_max * 128

    in_maps = []
    iota_np = np.broadcast_to(np.arange(128, dtype=np.float32),
                              (128, 128)).astype(BF16)
    xt_np = np.zeros((128, N_PAD), BF16)
    xt_np[:, :N_NODES] = x.T.astype(BF16)
    w1_np = W1.astype(BF16)
    w2_np = W2.astype(BF16)
    b1_np = b1.reshape(HID, 1).astype(np.float32)

    core_of_src = src // NSH
    for c in range(N_CORES):
        s_c, slot, n_c, win, counts, slot_of_dst = per_core[c]
        starts = np.zeros(N_WINDOWS, np.int64)
        starts[1:] = np.cumsum(counts)[:-1]
        pos_in_win = np.arange(len(s_c)) - starts[win]
        flat = win * slots_per_win + pos_in_win

        idx_pad = np.zeros(N_WINDOWS * slots_per_win, np.int32)
        drel_pad = np.full(N_WINDOWS * slots_per_win, -1.0, np.float32)
        norm_pad = np.zeros(N_WINDOWS * slots_per_win, np.float32)
        idx_pad[flat] = s_c
        drel_pad[flat] = (slot & 127).astype(np.float32)
        norm_pad[flat] = n_c

        def to_call_layout(a, dt):
            a = a.reshape(N_CALLS, CALL_WINDOWS, c_max, 128)
            return np.ascontiguousarray(a.transpose(0, 3, 1, 2)
                                        ).reshape(N_CALLS, 128, cc_n).astype(dt)

        idx_call = to_call_layout(idx_pad, np.int32)
        drel_call = to_call_layout(drel_pad, np.float32)
        norm_call = to_call_layout(norm_pad, np.float32)

        # ---- L2+pool folded weights (partitioned by src) ----
        m2 = core_of_src == c
        s2 = (src[m2] - c * NSH).astype(np.int64)
        g2 = gid[m2]
        n2 = norm[m2]
        wp = np.bincount(s2 * N_GRAPHS + g2, weights=n2,
                         minlength=NSH * N_GRAPHS).astype(np.float32)
        wp_full = np.zeros((WIN_SLOTS, N_GRAPHS), np.float32)
        wp_full[slot_of_dst, :] = wp.reshape(NSH, N_GRAPHS)

        in_maps.append({
            "xt": xt_np, "w1": w1_np, "w2": w2_np, "b1": b1_np,
            "iota": iota_np, "idx": idx_call, "drel": drel_call,
            "norm": norm_call, "wp": wp_full,
        })

    nc = _build(c_max)
    res = run_bass_kernel_spmd(nc, in_maps, core_ids=list(range(N_CORES)))
    LAST_RESULTS = res

    total = np.zeros((N_GRAPHS, OUT), np.float64)
    for c in range(N_CORES):
        total += np.asarray(res.results[c]["pool_part"], np.float64)
    cnt = np.bincount(batch, minlength=N_GRAPHS).astype(np.float64)
    out = total / np.maximum(cnt, 1.0)[:, None] + b2[None, :].astype(np.float64)
    return out.astype(np.float32)



# revision 14
# speedup vs baseline: 1.3054x; 1.3054x over previous
"""GCN 2-layer + global mean pool on 8 Trainium2 NeuronCores.

Strategy (sharding_hint: partition nodes+incident edges, replicate weights):
- h = x@W1 computed redundantly on every core (bf16, host passes x^T).
  Phase H batches 8 matmuls into one [128,512] PSUM bank and drains each
  bank with a single wide copy, alternating scalar/vector engines, so the
  copy engine no longer throttles the matmul stream.
- Layer-1 message passing partitioned by DST node: core c owns dst nodes
  [c*12500, (c+1)*12500). Edges sorted by dst-window (128 dsts/window),
  padded to C_MAX chunks of 128 edges per window. Messages gathered from
  the local h table via big indirect DMAs; scatter done as one-hot matmul
  (onehot[e, j] = (drel_e == j) * norm_e) accumulating in PSUM per window,
  output in transposed form [64f x 128dst] so bias+ReLU fuse into one
  scalar-engine activation with per-partition bias.
- h2 = relu(out1)@W2 per window on-core.
- Layer-2 message passing + mean-pool numerator folded into a dense matmul:
  pool[g,f] = sum_src Wp[src,g] * h2[src,f], where Wp[src,g] =
  sum_{edges e with src_e=src} norm_e * [batch[dst_e]=g] is precomputed on
  the host (layer-2 edges partitioned by SRC so h2 is local). Each core
  returns a partial [64,32]; host sums partials, divides by counts, adds b2.
"""

import numpy as np
import ml_dtypes

import concourse.bass as bass
import concourse.bacc as bacc
import concourse.tile as tile
import concourse.mybir as mybir
from concourse.bass_utils import run_bass_kernel_spmd
from concourse.masks import make_identity

BF16 = ml_dtypes.bfloat16

N_NODES = 100000
IN_DIM = 128
HID = 64
OUT = 32
N_GRAPHS = 64
N_CORES = 8
NSH = N_NODES // N_CORES          # 12500 dst nodes per core
N_WINDOWS = 100                   # 100 windows x 128 dst slots = 12800 >= 12500
WIN_SLOTS = N_WINDOWS * 128       # 12800
CALL_WINDOWS = 2                  # windows gathered per indirect DMA call
N_CALLS = N_WINDOWS // CALL_WINDOWS  # 50
HGROUP = 2048                     # nodes per h-phase group
N_HGROUPS = 49                    # 49*2048 = 100352
N_PAD = N_HGROUPS * HGROUP        # padded h-table rows

LAST_RESULTS = None


def _build(c_max):
    cc_n = CALL_WINDOWS * c_max   # chunks per gather call
    f32 = mybir.dt.float32
    bf16 = mybir.dt.bfloat16
    i32 = mybir.dt.int32
    RELU = mybir.ActivationFunctionType.Relu
    COPY = mybir.ActivationFunctionType.Copy

    nc = bacc.Bacc("TRN2", target_bir_lowering=False, debug=False,
                   enable_asserts=False, num_devices=N_CORES)

    xt = nc.dram_tensor("xt", [128, N_PAD], bf16, kind="ExternalInput")
    w1 = nc.dram_tensor("w1", [IN_DIM, HID], bf16, kind="ExternalInput")
    w2 = nc.dram_tensor("w2", [HID, OUT], bf16, kind="ExternalInput")
    b1 = nc.dram_tensor("b1", [HID, 1], f32, kind="ExternalInput")
    iota_in = nc.dram_tensor("iota", [128, 128], bf16, kind="ExternalInput")
    idx_in = nc.dram_tensor("idx", [N_CALLS, 128, cc_n], i32, kind="ExternalInput")
    drel_in = nc.dram_tensor("drel", [N_CALLS, 128, cc_n], f32, kind="ExternalInput")
    norm_in = nc.dram_tensor("norm", [N_CALLS, 128, cc_n], f32, kind="ExternalInput")
    wp_in = nc.dram_tensor("wp", [WIN_SLOTS, N_GRAPHS], f32, kind="ExternalInput")
    out_t = nc.dram_tensor("pool_part", [N_GRAPHS, OUT], f32, kind="ExternalOutput")

    with tile.TileContext(nc) as tc:
        with tc.tile_pool(name="const", bufs=1) as constp, \
             tc.tile_pool(name="stream", bufs=3) as sb, \
             tc.tile_pool(name="oh", bufs=6) as ohp, \
             tc.tile_pool(name="psh", bufs=2, space="PSUM") as psh, \
             tc.tile_pool(name="psr", bufs=3, space="PSUM") as psr, \
             tc.tile_pool(name="ps1", bufs=1, space="PSUM") as ps1, \
             tc.tile_pool(name="dram", bufs=1, space="DRAM") as dp:

            h_dram = dp.tile([N_PAD, HID], bf16)

            w1_sb = constp.tile([IN_DIM, HID], bf16)
            nc.sync.dma_start(w1_sb[:], w1[:])
            w2_sb = constp.tile([HID, OUT], bf16)
            nc.sync.dma_start(w2_sb[:], w2[:])
            b1_sb = constp.tile([HID, 1], f32)
            nc.sync.dma_start(b1_sb[:], b1[:])
            iota_sb = constp.tile([128, 128], bf16)
            nc.sync.dma_start(iota_sb[:], iota_in[:])
            ident = constp.tile([128, 128], f32)
            make_identity(nc, ident[:])
            h2_sb = constp.tile([128, N_WINDOWS * OUT], f32)
            wp_sb = constp.tile([128, N_WINDOWS * N_GRAPHS], f32)
            nc.sync.dma_start(
                wp_sb[:].rearrange("p (w g) -> p w g", g=N_GRAPHS),
                wp_in[:].rearrange("(w p) g -> p w g", p=128))

            # ---- phase H: h = x @ W1 for all nodes (redundant per core) ----
            # 16 matmuls per 2048-node group land in two [128,512] PSUM
            # banks; each bank drains with one wide copy (engines alternate)
            kpg = HGROUP // 128
            for g in range(N_HGROUPS):
                xt_t = sb.tile([128, HGROUP], bf16, tag="xt")
                nc.sync.dma_start(xt_t[:], xt[:, g * HGROUP:(g + 1) * HGROUP])
                hstage = sb.tile([128, kpg * HID], bf16, tag="hstage")
                for half in range(2):
                    ph = psh.tile([128, 8 * HID], f32, tag="ph")
                    for j in range(8):
                        k = half * 8 + j
                        nc.tensor.matmul(ph[:, j * HID:(j + 1) * HID],
                                         lhsT=xt_t[:, k * 128:(k + 1) * 128],
                                         rhs=w1_sb[:], start=True, stop=True)
                    dst = hstage[:, half * 8 * HID:(half + 1) * 8 * HID]
                    if half == 0:
                        nc.scalar.activation(dst, ph[:], COPY)
                    else:
                        nc.vector.tensor_copy(dst, ph[:])
                nc.sync.dma_start(
                    h_dram[g * HGROUP:(g + 1) * HGROUP, :].rearrange(
                        "(k p) f -> p k f", p=128),
                    hstage[:].rearrange("p (k f) -> p k f", f=HID))

            # ---- phase L1: gather + one-hot matmul scatter, then h2 ----
            for call in range(N_CALLS):
                idx_sb = sb.tile([128, cc_n], i32, tag="idx")
                nc.sync.dma_start(idx_sb[:], idx_in[call, :, :])
                drel_sb = sb.tile([128, cc_n], f32, tag="drel")
                nc.sync.dma_start(drel_sb[:], drel_in[call, :, :])
                norm_sb = sb.tile([128, cc_n], f32, tag="nrm")
                nc.sync.dma_start(norm_sb[:], norm_in[call, :, :])
                gath = sb.tile([128, cc_n * HID], bf16, tag="gath")
                nc.gpsimd.indirect_dma_start(
                    out=gath[:], out_offset=None,
                    in_=h_dram[:],
                    in_offset=bass.IndirectOffsetOnAxis(ap=idx_sb[:], axis=0))
                for wl in range(CALL_WINDOWS):
                    w = call * CALL_WINDOWS + wl
                    pR = psr.tile([HID, 128], f32, tag="pR")
                    for ci in range(c_max):
                        ch = wl * c_max + ci
                        oh = ohp.tile([128, 128], bf16, tag="oh")
                        nc.vector.tensor_scalar(
                            out=oh[:], in0=iota_sb[:],
                            scalar1=drel_sb[:, ch:ch + 1],
                            scalar2=norm_sb[:, ch:ch + 1],
                            op0=mybir.AluOpType.is_equal,
                            op1=mybir.AluOpType.mult)
                        nc.tensor.matmul(pR[:], lhsT=gath[:, ch * HID:(ch + 1) * HID],
                                         rhs=oh[:], start=(ci == 0),
                                         stop=(ci == c_max - 1))
                    relu_sb = sb.tile([HID, 128], bf16, tag="relu")
                    nc.scalar.activation(relu_sb[:], pR[:], RELU, bias=b1_sb[:, 0:1])
                    ph2 = ps1.tile([OUT, 128], f32, tag="ph2")
                    nc.tensor.matmul(ph2[:], lhsT=w2_sb[:], rhs=relu_sb[:],
                                     start=True, stop=True)
                    h2t = sb.tile([OUT, 128], f32, tag="h2t")
                    nc.scalar.activation(h2t[:], ph2[:], COPY)
                    ph2t = ps1.tile([128, OUT], f32, tag="ph2t")
                    nc.tensor.transpose(ph2t[:], h2t[:], ident[:OUT, :OUT])
                    nc.vector.tensor_copy(h2_sb[:, w * OUT:(w + 1) * OUT], ph2t[:])

            # ---- phase pool: pool_part = Wp^T @ h2 ----
            pp = ps1.tile([N_GRAPHS, OUT], f32, tag="pp")
            for w in range(N_WINDOWS):
                nc.tensor.matmul(pp[:],
                                 lhsT=wp_sb[:, w * N_GRAPHS:(w + 1) * N_GRAPHS],
                                 rhs=h2_sb[:, w * OUT:(w + 1) * OUT],
                                 start=(w == 0), stop=(w == N_WINDOWS - 1))
            outsb = sb.tile([N_GRAPHS, OUT], f32, tag="out")
            nc.vector.tensor_copy(outsb[:], pp[:])
            nc.sync.dma_start(out_t[:], outsb[:])

    nc.compile()
    return nc


def kernel(x, edge_index, batch, W1, b1, W2, b2):
    global LAST_RESULTS
    x = np.asarray(x, np.float32)
    src = np.asarray(edge_index[0], np.int64)
    dst = np.asarray(edge_index[1], np.int64)
    batch = np.asarray(batch, np.int64)
    W1 = np.asarray(W1, np.float32)
    b1 = np.asarray(b1, np.float32)
    W2 = np.asarray(W2, np.float32)
    b2 = np.asarray(b2, np.float32)

    loop = np.arange(N_NODES, dtype=np.int64)
    src = np.concatenate([src, loop])
    dst = np.concatenate([dst, loop])
    deg = np.bincount(dst, minlength=N_NODES).astype(np.float32)
    dinv = 1.0 / np.sqrt(np.maximum(deg, 1.0))
    norm = (dinv[src] * dinv[dst]).astype(np.float32)
    gid = batch[dst]                      # graph id of each edge's dst

    # ---- per-core L1 edge streams (partitioned by dst) ----
    core_of_dst = dst // NSH
    per_core = []
    c_max = 1
    for c in range(N_CORES):
        m = core_of_dst == c
        s_c = src[m].astype(np.int64)
        d_loc = (dst[m] - c * NSH).astype(np.int64)
        n_c = norm[m]
        # Balance window edge counts: assign dsts to window slots snake-wise
        # in descending-degree order, so every window gets ~equal edge work.
        degc = np.bincount(d_loc, minlength=NSH)
        rank_order = np.argsort(-degc, kind="stable")
        q, rem = np.divmod(np.arange(NSH), N_WINDOWS)
        win_of_rank = np.where(q % 2 == 0, rem, N_WINDOWS - 1 - rem)
        slot_of_rank = win_of_rank * 128 + q
        slot_of_dst = np.empty(NSH, np.int64)
        slot_of_dst[rank_order] = slot_of_rank
        slot = slot_of_dst[d_loc]
        win = slot >> 7
        order = np.argsort(win, kind="stable")
        s_c, slot, n_c, win = s_c[order], slot[order], n_c[order], win[order]
        counts = np.bincount(win, minlength=N_WINDOWS)
        c_max = max(c_max, int(np.ceil(counts.max() / 128)))
        per_core.append((s_c, slot, n_c, win, counts, slot_of_dst))

    cc_n = CALL_WINDOWS * c_max
    slots_per_win = c_max * 128

    in_maps = []
    iota_np = np.broadcast_to(np.arange(128, dtype=np.float32),
                              (128, 128)).astype(BF16)
    xt_np = np.zeros((128, N_PAD), BF16)
    xt_np[:, :N_NODES] = x.T.astype(BF16)
    w1_np = W1.astype(BF16)
    w2_np = W2.astype(BF16)
    b1_np = b1.reshape(HID, 1).astype(np.float32)

    core_of_src = src // NSH
    for c in range(N_CORES):
        s_c, slot, n_c, win, counts, slot_of_dst = per_core[c]
        starts = np.zeros(N_WINDOWS, np.int64)
        starts[1:] = np.cumsum(counts)[:-1]
        pos_in_win = np.arange(len(s_c)) - starts[win]
        flat = win * slots_per_win + pos_in_win

        idx_pad = np.zeros(N_WINDOWS * slots_per_win, np.int32)
        drel_pad = np.full(N_WINDOWS * slots_per_win, -1.0, np.float32)
        norm_pad = np.zeros(N_WINDOWS * slots_per_win, np.float32)
        idx_pad[flat] = s_c
        drel_pad[flat] = (slot & 127).astype(np.float32)
        norm_pad[flat] = n_c

        def to_call_layout(a, dt):
            a = a.reshape(N_CALLS, CALL_WINDOWS, c_max, 128)
            return np.ascontiguousarray(a.transpose(0, 3, 1, 2)
                                        ).reshape(N_CALLS, 128, cc_n).astype(dt)

        idx_call = to_call_layout(idx_pad, np.int32)
        drel_call = to_call_layout(drel_pad, np.float32)
        norm_call = to_call_layout(norm_pad, np.float32)

        # ---- L2+pool folded weights (partitioned by src) ----
        m2 = core_of_src == c
        s2 = (src[m2] - c * NSH).astype(np.int64)
        g2 = gid[m2]
        n2 = norm[m2]
        wp = np.bincount(s2 * N_GRAPHS + g2, weights=n2,
                         minlength=NSH * N_GRAPHS).astype(np.float32)
        wp_full = np.zeros((WIN_SLOTS, N_GRAPHS), np.float32)
        wp_full[slot_of_dst, :] = wp.reshape(NSH, N_GRAPHS)

        in_maps.append({
            "xt": xt_np, "w1": w1_np, "w2": w2_np, "b1": b1_np,
            "iota": iota_np, "idx": idx_call, "drel": drel_call,
            "norm": norm_call, "wp": wp_full,
        })

    nc = _build(c_max)
    res = run_bass_kernel_spmd(nc, in_maps, core_ids=list(range(N_CORES)))
    LAST_RESULTS = res

    total = np.zeros((N_GRAPHS, OUT), np.float64)
    for c in range(N_CORES):
        total += np.asarray(res.results[c]["pool_part"], np.float64)
    cnt = np.bincount(batch, minlength=N_GRAPHS).astype(np.float64)
    out = total / np.maximum(cnt, 1.0)[:, None] + b2[None, :].astype(np.float64)
    return out.astype(np.float32)
